# revision 27
# baseline (speedup 1.0000x reference)
"""Trainium2 Bass kernel for the gnn_message_passing problem.

Math (per edge e, side i):
  node_feat = l2norm(|dt|*w_time + b_time + gc*w_node + b_node)
  neigh_feat likewise per neighbor k
  att = tanh(node_feat@Wq + neigh_feat@Wk) . v_att
  score = leaky_relu(att + 2/(2+dt_neigh), 0.01)
  agg = sum_k (score*mask/n_neigh) * neigh_feat
  combined = [node_feat, agg]
  feat = sum_w exp(-0.5*bank_dt)*bank_mask * bank_feat + combined
  out = relu(feat @ weight.T)

Key structure exploited: every featurized vector lies in span{w_time, w_node,
b_time+b_node}, so node/neigh features are 3 scalars each. q+kk collapses to a
rank-6 combination of 6 fixed D-vectors; the "combined @ W.T" part of the
output collapses to a rank-6 combination of 6 fixed H-vectors. Only the
tanh( . ) . v contraction (E*2*K*D tanh evals) and the bank-feature reduction
touch O(E*K*D)-sized data on-device.

Sharding: pure data-parallel over E across 8 cores (one SPMD program).
"""

import numpy as np
import ml_dtypes

import concourse.bass as bass
import concourse.bacc as bacc
import concourse.mybir as mybir
import concourse.tile as tile
from concourse.bass_utils import run_bass_kernel_spmd

F32 = mybir.dt.float32
BF16 = mybir.dt.bfloat16
AF = mybir.ActivationFunctionType
OP = mybir.AluOpType

E, K, W, D, H = 4096, 32, 8, 128, 256
NCORES = 8
EC = E // NCORES          # 512 edges per core
POS = EC * 2              # 1024 (edge, side) positions per core
NT = POS // 128           # 8 position tiles of 128
D2 = 2 * D                # 256

# Per-tile attention geometry: 4096 arg columns (32 k * 128 pos), processed in
# 4 chunks of 1024 (ACT tanh granularity); each chunk = 2 matmuls of N=512.
CHUNKS = 4

USE_DMA_BCAST = True      # coef6 self-rows via broadcast-AP DMA


def _leaky(x, ns=0.01):
    return np.where(x >= 0, x, ns * x)


def _build_program(pp):
    """Build the SPMD single-core program. pp: dict of host-precomputed params."""
    nc = bacc.Bacc("TRN2", target_bir_lowering=False, debug=False)

    # ---- DRAM I/O (per core shard) ----
    # host-prepermuted: [128 p, (t k)] / [128 p, t]
    d_dtn = nc.dram_tensor("dtn_p", [128, 256], F32, kind="ExternalInput")
    d_gcn = nc.dram_tensor("gcn_p", [128, 256], F32, kind="ExternalInput")
    d_msk = nc.dram_tensor("mskn_p", [128, 256], F32, kind="ExternalInput")
    d_dts = nc.dram_tensor("dts_p", [128, 8], F32, kind="ExternalInput")
    d_gcs = nc.dram_tensor("gcs_p", [128, 8], F32, kind="ExternalInput")
    # bank decay inputs, host-prepermuted to [128 (po,wl), 64 (t,j,wh)]
    d_bdt = nc.dram_tensor("bdt_e", [128, 64], F32, kind="ExternalInput")
    d_bmsk = nc.dram_tensor("bmsk_e", [128, 64], F32, kind="ExternalInput")
    # host-prepermuted chunk-contiguous: chunk c=(t*4+j)*2+wh rows c*128..(c+1)*128
    d_bft = nc.dram_tensor("bft_p", [64 * 128, D2], F32, kind="ExternalInput")
    d_out = nc.dram_tensor("out", [POS, H], F32, kind="ExternalOutput")

    # ---- inline constants ----
    c_basis = nc.inline_tensor(pp["basis6att"], name="c_basis")    # [6,128] bf16
    c_b6h = nc.inline_tensor(pp["basis6H"], name="c_b6h")          # [8,256] f32
    c_v = nc.inline_tensor(pp["v32"], name="c_v")                  # [128,32] f32
    c_wT = nc.inline_tensor(pp["weightT"], name="c_wT")            # [256,256] f32
    c_dmask = nc.inline_tensor(pp["dmask"], name="c_dmask")        # [128,32] f32
    c_ident = nc.inline_tensor(pp["ident"], name="c_ident")        # [128,128] f32
    G = pp["gram"]  # 3x3 float

    from contextlib import ExitStack
    with tile.TileContext(nc) as tc, ExitStack() as ctx:
        cpool = ctx.enter_context(tc.tile_pool(name="consts", bufs=1))
        wpool = ctx.enter_context(tc.tile_pool(name="work", bufs=1))
        p_coef6 = ctx.enter_context(tc.tile_pool(name="coef6", bufs=2))
        p_tanh = ctx.enter_context(tc.tile_pool(name="tanh", bufs=3))
        p_attT = ctx.enter_context(tc.tile_pool(name="attT", bufs=3))
        p_featT = ctx.enter_context(tc.tile_pool(name="featT", bufs=16))
        p_bch = ctx.enter_context(tc.tile_pool(name="bch", bufs=3))
        p_mblk = ctx.enter_context(tc.tile_pool(name="mblk", bufs=2))
        p_out = ctx.enter_context(tc.tile_pool(name="outp", bufs=2))
        ps_arg = ctx.enter_context(tc.tile_pool(name="ps_arg", bufs=2, space="PSUM"))
        ps_att = ctx.enter_context(tc.tile_pool(name="ps_att", bufs=2, space="PSUM"))
        ps_bank = ctx.enter_context(tc.tile_pool(name="ps_bank", bufs=2, space="PSUM"))

        # ---- constants to SBUF ----
        cb_basis = cpool.tile([6, 128], BF16, name="cb_basis")
        nc.sync.dma_start(out=cb_basis, in_=c_basis[:, :])
        cb_b6h = cpool.tile([6, 256], F32, name="cb_b6h")
        nc.sync.dma_start(out=cb_b6h, in_=c_b6h[:, :])
        cb_v = cpool.tile([128, 32], F32, name="cb_v")
        nc.sync.dma_start(out=cb_v, in_=c_v[:, :])
        cb_wT0 = cpool.tile([128, 256], F32, name="cb_wT0")
        nc.sync.dma_start(out=cb_wT0, in_=c_wT[0:128, :])
        cb_wT1 = cpool.tile([128, 256], F32, name="cb_wT1")
        nc.sync.dma_start(out=cb_wT1, in_=c_wT[128:256, :])
        cb_dmask = cpool.tile([128, 32], F32, name="cb_dmask")
        nc.sync.dma_start(out=cb_dmask, in_=c_dmask[:, :])
        cb_id = cpool.tile([128, 128], F32, name="cb_id")
        nc.sync.dma_start(out=cb_id, in_=c_ident[:, :])

        # ---- stage 1: load per-position scalars, batched [128, 256(+8)] ----
        # layout: partition p, free (t, k): position = t*128 + p
        t_dtn = wpool.tile([128, 256], F32, name="t_dtn")
        nc.sync.dma_start(out=t_dtn[:, :], in_=d_dtn[:, :])
        t_m = wpool.tile([128, 256], F32, name="t_m")
        nc.sync.dma_start(out=t_m[:, :], in_=d_msk[:, :])
        # a_all/b_all: 264 cols = 256 neighbor + 8 self
        a_all = wpool.tile([128, 264], F32, name="a_all")
        b_all = wpool.tile([128, 264], F32, name="b_all")
        nc.sync.dma_start(out=a_all[:, 0:256], in_=d_dtn[:, :])
        nc.sync.dma_start(out=a_all[:, 256:264], in_=d_dts[:, :])
        nc.sync.dma_start(out=b_all[:, 0:256], in_=d_gcn[:, :])
        nc.sync.dma_start(out=b_all[:, 256:264], in_=d_gcs[:, :])

        # expanded bank decay inputs: [128 (po,wl), 64 (t,j,wh)]
        bdt_e = wpool.tile([128, 64], F32, name="bdt_e")
        bmsk_e = wpool.tile([128, 64], F32, name="bmsk_e")
        nc.sync.dma_start(out=bdt_e, in_=d_bdt[:, :])
        nc.sync.dma_start(out=bmsk_e, in_=d_bmsk[:, :])

        # ---- scalar math ----
        # a = |dt| = max(a, -a)
        nega = wpool.tile([128, 264], F32, name="nega")
        nc.vector.tensor_scalar(out=nega, in0=a_all, scalar1=-1.0, scalar2=None,
                                op0=OP.mult)
        nc.vector.tensor_tensor(out=a_all, in0=a_all, in1=nega, op=OP.max)
        aa = wpool.tile([128, 264], F32, name="aa")
        ab = wpool.tile([128, 264], F32, name="ab")
        bb = wpool.tile([128, 264], F32, name="bb")
        nc.vector.tensor_tensor(out=aa, in0=a_all, in1=a_all, op=OP.mult)
        nc.vector.tensor_tensor(out=ab, in0=a_all, in1=b_all, op=OP.mult)
        nc.vector.tensor_tensor(out=bb, in0=b_all, in1=b_all, op=OP.mult)
        n2 = wpool.tile([128, 264], F32, name="n2")
        # n2 = G00*aa + G22 ; += G11*bb ; += 2G02*a ; += 2G12*b ; += 2G01*ab
        nc.vector.tensor_scalar(out=n2, in0=aa, scalar1=float(G[0, 0]),
                                scalar2=float(G[2, 2]), op0=OP.mult, op1=OP.add)
        nc.vector.scalar_tensor_tensor(out=n2, in0=bb, scalar=float(G[1, 1]),
                                       in1=n2, op0=OP.mult, op1=OP.add)
        nc.vector.scalar_tensor_tensor(out=n2, in0=a_all, scalar=float(2 * G[0, 2]),
                                       in1=n2, op0=OP.mult, op1=OP.add)
        nc.vector.scalar_tensor_tensor(out=n2, in0=b_all, scalar=float(2 * G[1, 2]),
                                       in1=n2, op0=OP.mult, op1=OP.add)
        nc.vector.scalar_tensor_tensor(out=n2, in0=ab, scalar=float(2 * G[0, 1]),
                                       in1=n2, op0=OP.mult, op1=OP.add)
        nrm = wpool.tile([128, 264], F32, name="nrm")
        nc.scalar.activation(out=nrm, in_=n2, func=AF.Sqrt)   # sqrt table set
        nc.vector.tensor_scalar(out=nrm, in0=nrm, scalar1=1e-12, scalar2=None,
                                op0=OP.max)
        invn = wpool.tile([128, 264], F32, name="invn")
        nc.vector.reciprocal(out=invn, in_=nrm)
        alpha = wpool.tile([128, 264], F32, name="alpha")
        beta = wpool.tile([128, 264], F32, name="beta")
        nc.vector.tensor_tensor(out=alpha, in0=a_all, in1=invn, op=OP.mult)
        nc.vector.tensor_tensor(out=beta, in0=b_all, in1=invn, op=OP.mult)

        # time decay 2/(2+dt) on raw dt
        ts_t = wpool.tile([128, 256], F32, name="ts_t")
        nc.vector.tensor_scalar(out=ts_t, in0=t_dtn, scalar1=2.0, scalar2=None,
                                op0=OP.add)
        nc.vector.reciprocal(out=ts_t, in_=ts_t)
        nc.vector.tensor_scalar(out=ts_t, in0=ts_t, scalar1=2.0, scalar2=None,
                                op0=OP.mult)

        # n_neigh and mask/n_neigh
        nn = wpool.tile([128, 8], F32, name="nn")
        nc.vector.tensor_reduce(out=nn, in_=t_m.rearrange("p (t k) -> p t k", k=K),
                                axis=mybir.AxisListType.X, op=OP.add)
        nc.vector.tensor_scalar(out=nn, in0=nn, scalar1=1.0, scalar2=None,
                                op0=OP.max)
        innn = wpool.tile([128, 8], F32, name="innn")
        nc.vector.reciprocal(out=innn, in_=nn)
        mrec = wpool.tile([128, 256], F32, name="mrec")
        nc.vector.tensor_tensor(
            out=mrec.rearrange("p (t k) -> p t k", k=K),
            in0=t_m.rearrange("p (t k) -> p t k", k=K),
            in1=innn.unsqueeze(2).broadcast_to([128, 8, K]), op=OP.mult)

        # bank decay weights (dep on nrm forces exp-set load after sqrt-set use)
        zb = wpool.tile([128, 1], F32, name="zb")
        nc.vector.tensor_scalar(out=zb, in0=nrm[:, 0:1], scalar1=0.0, scalar2=None,
                                op0=OP.mult)
        bwe = wpool.tile([128, 64], F32, name="bwe")
        nc.scalar.activation(out=bwe, in_=bdt_e, func=AF.Exp, bias=zb, scale=-0.5)
        nc.vector.tensor_tensor(out=bwe, in0=bwe, in1=bmsk_e, op=OP.mult)

        # ---- stage 2: transposes for coef rows ----
        # pack self coefs [128, 24]: (coef, t) col = c*8+t
        packS = wpool.tile([128, 24], F32, name="packS")
        nc.vector.tensor_copy(out=packS[:, 0:8], in_=alpha[:, 256:264])
        nc.vector.tensor_copy(out=packS[:, 8:16], in_=beta[:, 256:264])
        nc.vector.tensor_copy(out=packS[:, 16:24], in_=invn[:, 256:264])
        pm = ps_att.tile([128, 512], F32, tag="psA", name="pm_selfT")
        nc.tensor.transpose(pm[0:24, 0:128], packS, cb_id)
        selfT = wpool.tile([32, 128], BF16, name="selfT")
        nc.vector.tensor_copy(out=selfT[0:24, :], in_=pm[0:24, 0:128])

        # neighbor coef transposes -> [128 (tq,k), 128 p] bf16, half h covers t=4h..4h+3
        coefT = []
        for (nm, srcT) in (("aT", alpha), ("bT", beta), ("gT", invn)):
            halves = []
            for h in range(2):
                pmx = ps_att.tile([128, 512], F32, tag="psA", name=f"pm_{nm}{h}")
                nc.tensor.transpose(pmx[0:128, 0:128],
                                    srcT[:, h * 128:(h + 1) * 128], cb_id)
                sb = wpool.tile([128, 128], BF16, name=f"{nm}{h}")
                nc.vector.tensor_copy(out=sb, in_=pmx[0:128, 0:128])
                halves.append(sb)
            coefT.append(halves)

        # persistent per-core tensors
        att_a = wpool.tile([128, 256], F32, name="att_a")
        coefF6 = wpool.tile([6, 8 * 128], F32, name="coefF6")
        ABC = wpool.tile([128, 24], F32, name="ABC")  # cols c*8+t

        featT_sb = [[None] * 2 for _ in range(NT)]

        # ---- per-tile pipelines ----
        for t in range(NT):
            # coef6 rhs [8, 4096] bf16, col = k*128 + p
            coef6 = p_coef6.tile([6, 4096], BF16, tag="coef6", name=f"coef6_{t}")
            for c in range(3):
                r = c * 8 + t
                nc.sync.dma_start(
                    out=coef6[c:c + 1, :],
                    in_=selfT[r:r + 1, :].unsqueeze(1).broadcast_to([1, K, 128]))
            for c in range(3):
                src = coefT[c][t // 4]
                nc.sync.dma_start(
                    out=coef6[3 + c:4 + c, :],
                    in_=src[(t % 4) * 32:(t % 4) * 32 + 32, :])

            # attention chunks
            attT = p_attT.tile([32, 128], F32, tag="attT", name=f"attT_{t}")
            for cc in range(CHUNKS):
                pa = ps_arg.tile([128, 1024], F32, tag="psarg", name=f"pa_{t}_{cc}")
                for mm in range(2):
                    nc.tensor.matmul(
                        pa[:, mm * 512:(mm + 1) * 512], lhsT=cb_basis,
                        rhs=coef6[:, cc * 1024 + mm * 512: cc * 1024 + (mm + 1) * 512],
                        start=True, stop=True)
                th = p_tanh.tile([128, 1024], F32, tag="tanh", name=f"th_{t}_{cc}")
                nc.scalar.activation(out=th, in_=pa, func=AF.Tanh)
                if cc % 2 == 0:
                    pv = ps_att.tile([128, 512], F32, tag="psA", name=f"pv_{t}_{cc}")
                for mm in range(2):
                    q = (cc % 2) * 2 + mm
                    nc.tensor.matmul(pv[32 * q:32 * (q + 1), :], lhsT=cb_v,
                                     rhs=th[:, mm * 512:(mm + 1) * 512],
                                     start=True, stop=True,
                                     tile_position=(0, 32 * q))
                if cc % 2 == 1:
                    b = cc // 2
                    ast = p_mblk.tile([128, 512], F32, tag="astage",
                                      name=f"ast_{t}_{cc}")
                    nc.vector.tensor_copy(out=ast[:, :], in_=pv[:, :])
                    nc.sync.dma_start(
                        out=attT[16 * b:16 * (b + 1), :],
                        in_=ast.rearrange("(q r) (kl p) -> q r kl p", r=32, p=128)[:, 0])

            # att back to [pos, k] layout
            pmx = ps_att.tile([128, 512], F32, tag="psA", name=f"pm_att_{t}")
            nc.tensor.transpose(pmx[0:128, 0:32], attT, cb_id[0:32, 0:32])
            nc.vector.tensor_copy(out=att_a[:, 32 * t:32 * (t + 1)],
                                  in_=pmx[0:128, 0:32])

            # bank feature reduction -> featT (layout B) via selector matmuls
            mb = p_mblk.tile([128, 256], F32, tag="mblk", name=f"mb_{t}")
            nc.vector.tensor_tensor(
                out=mb.rearrange("r (b c) -> r b c", c=32),
                in0=cb_dmask.unsqueeze(1).broadcast_to([128, 8, 32]),
                in1=bwe[:, t * 8:(t + 1) * 8].unsqueeze(2).broadcast_to([128, 8, 32]),
                op=OP.mult)
            fpA = ps_bank.tile([128, 256], F32, tag="psB", name=f"fpA_{t}")
            for j in range(4):
                for wh in range(2):
                    c = (t * 4 + j) * 2 + wh
                    bc = p_bch.tile([128, 256], F32, tag="bch", name=f"bc_{t}_{j}_{wh}")
                    nc.gpsimd.dma_start(
                        out=bc[:, :], in_=d_bft[c * 128:(c + 1) * 128, :])
                    nc.tensor.matmul(
                        fpA[32 * j:32 * (j + 1), :],
                        lhsT=mb[:, 32 * (2 * j + wh):32 * (2 * j + wh + 1)],
                        rhs=bc[:, :],
                        start=(wh == 0), stop=(wh == 1),
                        tile_position=(0, 32 * j))
            bkA = p_mblk.tile([128, 256], F32, tag="bkA", name=f"bkA_{t}")
            nc.vector.tensor_copy(out=bkA, in_=fpA)
            for h in range(2):
                pmb = ps_att.tile([128, 512], F32, tag="psA", name=f"pmb_{t}_{h}")
                nc.tensor.transpose(pmb[0:128, 0:128],
                                    bkA[:, h * 128:(h + 1) * 128], cb_id)
                fsb = p_featT.tile([128, 128], F32, tag="featT", name=f"fT_{t}_{h}")
                nc.vector.tensor_copy(out=fsb, in_=pmb[0:128, 0:128])
                featT_sb[t][h] = fsb

        # ---- stage 5: score + agg coefficients (batched) ----
        sc = wpool.tile([128, 256], F32, name="sc")
        nc.vector.tensor_tensor(out=sc, in0=att_a, in1=ts_t, op=OP.add)
        sc2 = wpool.tile([128, 256], F32, name="sc2")
        nc.vector.tensor_scalar(out=sc2, in0=sc, scalar1=0.01, scalar2=None,
                                op0=OP.mult)
        nc.vector.tensor_tensor(out=sc, in0=sc, in1=sc2, op=OP.max)
        wgt = wpool.tile([128, 256], F32, name="wgt")
        nc.vector.tensor_tensor(out=wgt, in0=sc, in1=mrec, op=OP.mult)
        prod = wpool.tile([128, 256], F32, name="prod")
        for c, src in enumerate((alpha, beta, invn)):
            nc.vector.tensor_tensor(out=prod, in0=wgt, in1=src[:, 0:256], op=OP.mult)
            nc.vector.tensor_reduce(out=ABC[:, c * 8:(c + 1) * 8],
                                    in_=prod.rearrange("p (t k) -> p t k", k=K),
                                    axis=mybir.AxisListType.X, op=OP.add)

        # pack final rank-6 coefs: col = c*8 + t, rows: (as,bs,gs,A,B,C)
        packF = wpool.tile([128, 48], F32, name="packF")
        for c, src in ((0, alpha[:, 256:264]), (1, beta[:, 256:264]),
                       (2, invn[:, 256:264]), (3, ABC[:, 0:8]),
                       (4, ABC[:, 8:16]), (5, ABC[:, 16:24])):
            nc.vector.tensor_copy(out=packF[:, c * 8:(c + 1) * 8], in_=src)
        pmf = ps_att.tile([128, 512], F32, tag="psA", name="pm_packF")
        nc.tensor.transpose(pmf[0:48, 0:128], packF, cb_id)
        pFT = wpool.tile([48, 128], F32, name="pFT")
        nc.vector.tensor_copy(out=pFT, in_=pmf[0:48, 0:128])
        for c in range(6):
            nc.sync.dma_start(out=coefF6[c:c + 1, :],
                              in_=pFT[c * 8:(c + 1) * 8, :])

        # ---- stage 6: final matmuls + relu + store ----
        for t in range(NT):
            po = ps_att.tile([128, 512], F32, tag="psA", name=f"po_{t}")
            nc.tensor.matmul(po[:, 0:256], lhsT=featT_sb[t][0], rhs=cb_wT0,
                             start=True, stop=False)
            nc.tensor.matmul(po[:, 0:256], lhsT=featT_sb[t][1], rhs=cb_wT1,
                             start=False, stop=False)
            nc.tensor.matmul(po[:, 0:256], lhsT=coefF6[:, t * 128:(t + 1) * 128],
                             rhs=cb_b6h, start=False, stop=True)
            ot = p_out.tile([128, 256], F32, tag="outp", name=f"ot_{t}")
            nc.vector.tensor_scalar(out=ot, in0=po[:, 0:256], scalar1=0.0,
                                    scalar2=None, op0=OP.max)
            nc.sync.dma_start(out=d_out[t * 128:(t + 1) * 128, :], in_=ot)

    nc.compile()
    return nc


def _host_params(w_time, b_time, w_node, b_node, Wq, Wk, v_att, weight):
    f32 = np.float32
    w_time = np.asarray(w_time, f32)
    w_node = np.asarray(w_node, f32)
    bsum = np.asarray(b_time, f32) + np.asarray(b_node, f32)
    Wq = np.asarray(Wq, f32)
    Wk = np.asarray(Wk, f32)
    v = np.asarray(v_att, f32)
    weight = np.asarray(weight, f32)

    basis3 = np.stack([w_time, w_node, bsum])                  # [3, D]
    gram = basis3 @ basis3.T                                   # [3, 3]
    basis6att = np.zeros((6, D), f32)
    basis6att[0:3] = basis3 @ Wq
    basis6att[3:6] = basis3 @ Wk
    basis6H = np.zeros((6, H), f32)
    basis6H[0:3] = basis3 @ weight[:, :D].T
    basis6H[3:6] = basis3 @ weight[:, D:].T
    dmask = np.zeros((128, 32), f32)
    dmask[np.arange(128), np.arange(128) // 4] = 1.0
    return {
        "basis6att": basis6att.astype(ml_dtypes.bfloat16),
        "basis6H": basis6H,
        "v32": np.ascontiguousarray(np.tile(v.reshape(D, 1), (1, 32))),
        "weightT": np.ascontiguousarray(weight.T),
        "dmask": dmask,
        "ident": np.eye(128, dtype=f32),
        "gram": gram.astype(np.float64),
    }


def _perm_tk(x):
    # [EC,2,K] -> [128 p, (t k)]
    return np.ascontiguousarray(
        x.reshape(NT, 128, K).transpose(1, 0, 2).reshape(128, NT * K))


def _perm_t(x):
    # [EC,2] -> [128 p, t]
    return np.ascontiguousarray(x.reshape(NT, 128).T)


def _perm_bft(x):
    # [EC,2,W,D2] -> rows ((t j wh),(po wl)) x D2
    x = x.reshape(NT, 4, 32, 2, 4, D2)       # t j po wh wl d
    x = x.transpose(0, 1, 3, 2, 4, 5)        # t j wh po wl d
    return np.ascontiguousarray(x.reshape(64 * 128, D2))


def _expand_bank(x):
    # [EC,2,W] -> [128 (po,wl), 64 (t,j,wh)]: x[t*128+j*32+po, wh*4+wl]
    x = x.reshape(NT, 4, 32, 2, 4)          # t j po wh wl
    x = x.transpose(2, 4, 0, 1, 3)          # po wl t j wh
    return np.ascontiguousarray(x.reshape(128, 64))


def _shard_inputs(inputs):
    f32 = np.float32
    ins = []
    for c in range(NCORES):
        sl = slice(c * EC, (c + 1) * EC)
        ins.append({
            "dtn_p": _perm_tk(np.asarray(inputs["dt_neigh"][sl], f32)),
            "gcn_p": _perm_tk(np.asarray(inputs["gc_neigh"][sl], f32)),
            "mskn_p": _perm_tk(
                np.asarray(inputs["neigh_mask"][sl]).astype(f32)),
            "dts_p": _perm_t(np.asarray(inputs["dt_self"][sl], f32)),
            "gcs_p": _perm_t(np.asarray(inputs["gc_self"][sl], f32)),
            "bdt_e": _expand_bank(np.asarray(inputs["bank_dt"][sl], f32)),
            "bmsk_e": _expand_bank(
                np.asarray(inputs["bank_mask"][sl]).astype(f32)),
            "bft_p": _perm_bft(np.asarray(inputs["bank_feat"][sl], f32)),
        })
    return ins


_LAST_RESULT = {}


def kernel(**inputs):
    pp = _host_params(inputs["w_time"], inputs["b_time"], inputs["w_node"],
                      inputs["b_node"], inputs["Wq"], inputs["Wk"],
                      inputs["v_att"], inputs["weight"])
    nc = _build_program(pp)
    in_maps = _shard_inputs(inputs)
    import os
    trace = bool(int(os.environ.get("KBENCH_TRACE", "0")))
    res = run_bass_kernel_spmd(nc, in_maps, core_ids=list(range(NCORES)),
                               trace=trace)
    _LAST_RESULT["res"] = res
    outs = [res.results[c]["out"].reshape(EC, 2, H) for c in range(NCORES)]
    return np.ascontiguousarray(np.concatenate(outs, axis=0))


# revision 28
# speedup vs baseline: 1.2800x; 1.2800x over previous
"""Trainium2 Bass kernel for the gnn_message_passing problem.

Math (per edge e, side i):
  node_feat = l2norm(|dt|*w_time + b_time + gc*w_node + b_node)
  neigh_feat likewise per neighbor k
  att = tanh(node_feat@Wq + neigh_feat@Wk) . v_att
  score = leaky_relu(att + 2/(2+dt_neigh), 0.01)
  agg = sum_k (score*mask/n_neigh) * neigh_feat
  combined = [node_feat, agg]
  feat = sum_w exp(-0.5*bank_dt)*bank_mask * bank_feat + combined
  out = relu(feat @ weight.T)

Key structure exploited: every featurized vector lies in span{w_time, w_node,
b_time+b_node}, so node/neigh features are 3 scalars each. q+kk collapses to a
rank-6 combination of 6 fixed D-vectors; the "combined @ W.T" part of the
output collapses to a rank-6 combination of 6 fixed H-vectors. Only the
tanh( . ) . v contraction (E*2*K*D tanh evals) and the bank-feature reduction
touch O(E*K*D)-sized data on-device.

Sharding: pure data-parallel over E across 8 cores (one SPMD program).
"""

import numpy as np
import ml_dtypes

import concourse.bass as bass
import concourse.bacc as bacc
import concourse.mybir as mybir
import concourse.tile as tile
from concourse.bass_utils import run_bass_kernel_spmd

F32 = mybir.dt.float32
BF16 = mybir.dt.bfloat16
AF = mybir.ActivationFunctionType
OP = mybir.AluOpType

E, K, W, D, H = 4096, 32, 8, 128, 256
NCORES = 8
EC = E // NCORES          # 512 edges per core
POS = EC * 2              # 1024 (edge, side) positions per core
NT = POS // 128           # 8 position tiles of 128
D2 = 2 * D                # 256

# Per-tile attention geometry: 4096 arg columns (32 k * 128 pos), processed in
# 4 chunks of 1024 (ACT tanh granularity); each chunk = 2 matmuls of N=512.
CHUNKS = 4

USE_DMA_BCAST = True      # coef6 self-rows via broadcast-AP DMA


def _leaky(x, ns=0.01):
    return np.where(x >= 0, x, ns * x)


def _build_program(pp):
    """Build the SPMD single-core program. pp: dict of host-precomputed params."""
    nc = bacc.Bacc("TRN2", target_bir_lowering=False, debug=False)

    # ---- DRAM I/O (per core shard) ----
    # host-prepermuted: [128 p, (t k)] / [128 p, t]
    d_dtn = nc.dram_tensor("dtn_p", [128, 256], F32, kind="ExternalInput")
    d_gcn = nc.dram_tensor("gcn_p", [128, 256], F32, kind="ExternalInput")
    d_msk = nc.dram_tensor("mskn_p", [128, 256], F32, kind="ExternalInput")
    d_dts = nc.dram_tensor("dts_p", [128, 8], F32, kind="ExternalInput")
    d_gcs = nc.dram_tensor("gcs_p", [128, 8], F32, kind="ExternalInput")
    # bank decay inputs, host-prepermuted to [128 (po,wl), 64 (t,j,wh)]
    d_bdt = nc.dram_tensor("bdt_e", [128, 64], F32, kind="ExternalInput")
    d_bmsk = nc.dram_tensor("bmsk_e", [128, 64], F32, kind="ExternalInput")
    # host-prepermuted chunk-contiguous: chunk c=(t*4+j)*2+wh rows c*128..(c+1)*128
    d_bft = nc.dram_tensor("bft_p", [64 * 128, D2], F32, kind="ExternalInput")
    d_out = nc.dram_tensor("out", [POS, H], F32, kind="ExternalOutput")

    # ---- inline constants ----
    c_basis = nc.inline_tensor(pp["basis6att"], name="c_basis")    # [6,128] bf16
    c_b6h = nc.inline_tensor(pp["basis6H"], name="c_b6h")          # [8,256] f32
    c_v = nc.inline_tensor(pp["v32"], name="c_v")                  # [128,32] f32
    c_wT = nc.inline_tensor(pp["weightT"], name="c_wT")            # [256,256] f32
    c_dmask = nc.inline_tensor(pp["dmask"], name="c_dmask")        # [128,32] f32
    c_ident = nc.inline_tensor(pp["ident"], name="c_ident")        # [128,128] f32
    G = pp["gram"]  # 3x3 float

    from contextlib import ExitStack
    with tile.TileContext(nc) as tc, ExitStack() as ctx:
        cpool = ctx.enter_context(tc.tile_pool(name="consts", bufs=1))
        wpool = ctx.enter_context(tc.tile_pool(name="work", bufs=1))
        p_coef6 = ctx.enter_context(tc.tile_pool(name="coef6", bufs=2))
        p_tanh = ctx.enter_context(tc.tile_pool(name="tanh", bufs=3))
        p_attT = ctx.enter_context(tc.tile_pool(name="attT", bufs=3))
        p_featT = ctx.enter_context(tc.tile_pool(name="featT", bufs=16))
        p_bch = ctx.enter_context(tc.tile_pool(name="bch", bufs=3))
        p_mblk = ctx.enter_context(tc.tile_pool(name="mblk", bufs=2))
        p_out = ctx.enter_context(tc.tile_pool(name="outp", bufs=2))
        ps_arg = ctx.enter_context(tc.tile_pool(name="ps_arg", bufs=2, space="PSUM"))
        ps_att = ctx.enter_context(tc.tile_pool(name="ps_att", bufs=2, space="PSUM"))
        ps_bank = ctx.enter_context(tc.tile_pool(name="ps_bank", bufs=2, space="PSUM"))

        # ---- constants to SBUF ----
        cb_basis = cpool.tile([6, 128], BF16, name="cb_basis")
        nc.sync.dma_start(out=cb_basis, in_=c_basis[:, :])
        cb_b6h = cpool.tile([6, 256], F32, name="cb_b6h")
        nc.sync.dma_start(out=cb_b6h, in_=c_b6h[:, :])
        cb_v = cpool.tile([128, 32], F32, name="cb_v")
        nc.sync.dma_start(out=cb_v, in_=c_v[:, :])
        cb_wT0 = cpool.tile([128, 256], F32, name="cb_wT0")
        nc.sync.dma_start(out=cb_wT0, in_=c_wT[0:128, :])
        cb_wT1 = cpool.tile([128, 256], F32, name="cb_wT1")
        nc.sync.dma_start(out=cb_wT1, in_=c_wT[128:256, :])
        cb_dmask = cpool.tile([128, 32], F32, name="cb_dmask")
        nc.sync.dma_start(out=cb_dmask, in_=c_dmask[:, :])
        cb_id = cpool.tile([128, 128], F32, name="cb_id")
        nc.sync.dma_start(out=cb_id, in_=c_ident[:, :])

        # ---- stage 1: load per-position scalars, batched [128, 256(+8)] ----
        # layout: partition p, free (t, k): position = t*128 + p
        t_dtn = wpool.tile([128, 256], F32, name="t_dtn")
        nc.sync.dma_start(out=t_dtn[:, :], in_=d_dtn[:, :])
        t_m = wpool.tile([128, 256], F32, name="t_m")
        nc.sync.dma_start(out=t_m[:, :], in_=d_msk[:, :])
        # a_all/b_all: 264 cols = 256 neighbor + 8 self
        a_all = wpool.tile([128, 264], F32, name="a_all")
        b_all = wpool.tile([128, 264], F32, name="b_all")
        nc.sync.dma_start(out=a_all[:, 0:256], in_=d_dtn[:, :])
        nc.sync.dma_start(out=a_all[:, 256:264], in_=d_dts[:, :])
        nc.sync.dma_start(out=b_all[:, 0:256], in_=d_gcn[:, :])
        nc.sync.dma_start(out=b_all[:, 256:264], in_=d_gcs[:, :])

        # expanded bank decay inputs: [128 (po,wl), 64 (t,j,wh)]
        bdt_e = wpool.tile([128, 64], F32, name="bdt_e")
        bmsk_e = wpool.tile([128, 64], F32, name="bmsk_e")
        nc.sync.dma_start(out=bdt_e, in_=d_bdt[:, :])
        nc.sync.dma_start(out=bmsk_e, in_=d_bmsk[:, :])

        # ---- scalar math ----
        # a = |dt| = max(a, -a)
        nega = wpool.tile([128, 264], F32, name="nega")
        nc.vector.tensor_scalar(out=nega, in0=a_all, scalar1=-1.0, scalar2=None,
                                op0=OP.mult)
        nc.vector.tensor_tensor(out=a_all, in0=a_all, in1=nega, op=OP.max)
        aa = wpool.tile([128, 264], F32, name="aa")
        ab = wpool.tile([128, 264], F32, name="ab")
        bb = wpool.tile([128, 264], F32, name="bb")
        nc.vector.tensor_tensor(out=aa, in0=a_all, in1=a_all, op=OP.mult)
        nc.vector.tensor_tensor(out=ab, in0=a_all, in1=b_all, op=OP.mult)
        nc.vector.tensor_tensor(out=bb, in0=b_all, in1=b_all, op=OP.mult)
        n2 = wpool.tile([128, 264], F32, name="n2")
        # n2 = G00*aa + G22 ; += G11*bb ; += 2G02*a ; += 2G12*b ; += 2G01*ab
        nc.vector.tensor_scalar(out=n2, in0=aa, scalar1=float(G[0, 0]),
                                scalar2=float(G[2, 2]), op0=OP.mult, op1=OP.add)
        nc.vector.scalar_tensor_tensor(out=n2, in0=bb, scalar=float(G[1, 1]),
                                       in1=n2, op0=OP.mult, op1=OP.add)
        nc.vector.scalar_tensor_tensor(out=n2, in0=a_all, scalar=float(2 * G[0, 2]),
                                       in1=n2, op0=OP.mult, op1=OP.add)
        nc.vector.scalar_tensor_tensor(out=n2, in0=b_all, scalar=float(2 * G[1, 2]),
                                       in1=n2, op0=OP.mult, op1=OP.add)
        nc.vector.scalar_tensor_tensor(out=n2, in0=ab, scalar=float(2 * G[0, 1]),
                                       in1=n2, op0=OP.mult, op1=OP.add)
        nrm = wpool.tile([128, 264], F32, name="nrm")
        nc.scalar.activation(out=nrm, in_=n2, func=AF.Sqrt)   # sqrt table set
        nc.vector.tensor_scalar(out=nrm, in0=nrm, scalar1=1e-12, scalar2=None,
                                op0=OP.max)
        invn = wpool.tile([128, 264], F32, name="invn")
        nc.vector.reciprocal(out=invn, in_=nrm)
        alpha = wpool.tile([128, 264], F32, name="alpha")
        beta = wpool.tile([128, 264], F32, name="beta")
        nc.vector.tensor_tensor(out=alpha, in0=a_all, in1=invn, op=OP.mult)
        nc.vector.tensor_tensor(out=beta, in0=b_all, in1=invn, op=OP.mult)

        # time decay 2/(2+dt) on raw dt
        ts_t = wpool.tile([128, 256], F32, name="ts_t")
        nc.vector.tensor_scalar(out=ts_t, in0=t_dtn, scalar1=2.0, scalar2=None,
                                op0=OP.add)
        nc.vector.reciprocal(out=ts_t, in_=ts_t)
        nc.vector.tensor_scalar(out=ts_t, in0=ts_t, scalar1=2.0, scalar2=None,
                                op0=OP.mult)

        # n_neigh and mask/n_neigh
        nn = wpool.tile([128, 8], F32, name="nn")
        nc.vector.tensor_reduce(out=nn, in_=t_m.rearrange("p (t k) -> p t k", k=K),
                                axis=mybir.AxisListType.X, op=OP.add)
        nc.vector.tensor_scalar(out=nn, in0=nn, scalar1=1.0, scalar2=None,
                                op0=OP.max)
        innn = wpool.tile([128, 8], F32, name="innn")
        nc.vector.reciprocal(out=innn, in_=nn)
        mrec = wpool.tile([128, 256], F32, name="mrec")
        nc.vector.tensor_tensor(
            out=mrec.rearrange("p (t k) -> p t k", k=K),
            in0=t_m.rearrange("p (t k) -> p t k", k=K),
            in1=innn.unsqueeze(2).broadcast_to([128, 8, K]), op=OP.mult)

        # bank decay weights (dep on nrm forces exp-set load after sqrt-set use)
        zb = wpool.tile([128, 1], F32, name="zb")
        nc.vector.tensor_scalar(out=zb, in0=nrm[:, 0:1], scalar1=0.0, scalar2=None,
                                op0=OP.mult)
        bwe = wpool.tile([128, 64], F32, name="bwe")
        nc.scalar.activation(out=bwe, in_=bdt_e, func=AF.Exp, bias=zb, scale=-0.5)
        nc.vector.tensor_tensor(out=bwe, in0=bwe, in1=bmsk_e, op=OP.mult)

        # ---- stage 2: transposes for coef rows ----
        # pack self coefs [128, 24]: (coef, t) col = c*8+t
        packS = wpool.tile([128, 24], F32, name="packS")
        nc.vector.tensor_copy(out=packS[:, 0:8], in_=alpha[:, 256:264])
        nc.vector.tensor_copy(out=packS[:, 8:16], in_=beta[:, 256:264])
        nc.vector.tensor_copy(out=packS[:, 16:24], in_=invn[:, 256:264])
        pm = ps_att.tile([128, 512], F32, tag="psA", name="pm_selfT")
        nc.tensor.transpose(pm[0:24, 0:128], packS, cb_id)
        selfT = wpool.tile([32, 128], BF16, name="selfT")
        nc.vector.tensor_copy(out=selfT[0:24, :], in_=pm[0:24, 0:128])

        # neighbor coef transposes -> [128 (tq,k), 128 p] bf16, half h covers t=4h..4h+3
        coefT = []
        for (nm, srcT) in (("aT", alpha), ("bT", beta), ("gT", invn)):
            halves = []
            for h in range(2):
                pmx = ps_att.tile([128, 512], F32, tag="psA", name=f"pm_{nm}{h}")
                nc.tensor.transpose(pmx[0:128, 0:128],
                                    srcT[:, h * 128:(h + 1) * 128], cb_id)
                sb = wpool.tile([128, 128], BF16, name=f"{nm}{h}")
                nc.vector.tensor_copy(out=sb, in_=pmx[0:128, 0:128])
                halves.append(sb)
            coefT.append(halves)

        # persistent per-core tensors
        att_a = wpool.tile([128, 256], F32, name="att_a")
        coefF6 = wpool.tile([6, 8 * 128], F32, name="coefF6")
        ABC = wpool.tile([128, 24], F32, name="ABC")  # cols c*8+t

        featT_sb = [[None] * 2 for _ in range(NT)]

        # ---- software-pipelined global chunk loop ----
        # Per global chunk gc (t=gc//4, cc=gc%4) the PE queue sees:
        #   arg(gc) -> vdot(gc-1) -> bank pair (t, j=cc)
        # so vdot(gc-1) is reached only after tanh(gc-1) had a full chunk of
        # time, and bank matmuls fill the remaining ACT-paced slack.
        def build_coef6(t):
            c6 = p_coef6.tile([6, 4096], BF16, tag="coef6", name=f"coef6_{t}")
            for c in range(3):
                r = c * 8 + t
                nc.sync.dma_start(
                    out=c6[c:c + 1, :],
                    in_=selfT[r:r + 1, :].unsqueeze(1).broadcast_to([1, K, 128]))
            for c in range(3):
                src = coefT[c][t // 4]
                nc.sync.dma_start(
                    out=c6[3 + c:4 + c, :],
                    in_=src[(t % 4) * 32:(t % 4) * 32 + 32, :])
            return c6

        def build_mb(t):
            mb = p_mblk.tile([128, 256], F32, tag="mblk", name=f"mb_{t}")
            nc.vector.tensor_tensor(
                out=mb.rearrange("r (b c) -> r b c", c=32),
                in0=cb_dmask.unsqueeze(1).broadcast_to([128, 8, 32]),
                in1=bwe[:, t * 8:(t + 1) * 8].unsqueeze(2).broadcast_to(
                    [128, 8, 32]),
                op=OP.mult)
            return mb

        def load_bc(gidx):
            bc = p_bch.tile([128, 256], F32, tag="bch", name=f"bc_{gidx}")
            nc.gpsimd.dma_start(out=bc[:, :],
                                in_=d_bft[gidx * 128:(gidx + 1) * 128, :])
            return bc

        coef6_t = build_coef6(0)
        mb_t = build_mb(0)
        bc_pend = [load_bc(0), load_bc(1)]
        pend_vdot = None        # (th, pv, cc)
        state = {}              # per-tile live tiles

        def emit_vdot(th, pv, cc, t):
            for mm in range(2):
                q = (cc % 2) * 2 + mm
                nc.tensor.matmul(pv[32 * q:32 * (q + 1), :], lhsT=cb_v,
                                 rhs=th[:, mm * 512:(mm + 1) * 512],
                                 start=True, stop=True,
                                 tile_position=(0, 32 * q))
            if cc % 2 == 1:
                b = cc // 2
                ast = p_mblk.tile([128, 512], F32, tag="astage",
                                  name=f"ast_{t}_{cc}")
                nc.vector.tensor_copy(out=ast[:, :], in_=pv[:, :])
                attT = state[t]["attT"]
                nc.sync.dma_start(
                    out=attT[16 * b:16 * (b + 1), :],
                    in_=ast.rearrange("(q r) (kl p) -> q r kl p",
                                      r=32, p=128)[:, 0])
            if cc == 3:
                # att for tile t complete -> back to [pos, k] layout
                attT = state[t]["attT"]
                pmx = ps_bank.tile([128, 256], F32, tag="psB", name=f"pmx_{t}")
                nc.tensor.transpose(pmx[0:128, 0:32], attT, cb_id[0:32, 0:32])
                nc.vector.tensor_copy(out=att_a[:, 32 * t:32 * (t + 1)],
                                      in_=pmx[0:128, 0:32])

        for gc in range(NT * CHUNKS):
            t, cc = divmod(gc, CHUNKS)
            if cc == 0:
                state[t] = {
                    "coef6": coef6_t, "mb": mb_t,
                    "attT": p_attT.tile([32, 128], F32, tag="attT",
                                        name=f"attT_{t}"),
                    "fpA": ps_bank.tile([128, 256], F32, tag="psB",
                                        name=f"fpA_{t}"),
                }
                if t + 1 < NT:     # prefetch next tile's inputs
                    coef6_t = build_coef6(t + 1)
                    mb_t = build_mb(t + 1)
            st = state[t]
            # arg matmuls + tanh for this chunk
            pa = ps_arg.tile([128, 1024], F32, tag="psarg", name=f"pa_{gc}")
            for mm in range(2):
                nc.tensor.matmul(
                    pa[:, mm * 512:(mm + 1) * 512], lhsT=cb_basis,
                    rhs=st["coef6"][:, cc * 1024 + mm * 512:
                                    cc * 1024 + (mm + 1) * 512],
                    start=True, stop=True)
            th = p_tanh.tile([128, 1024], F32, tag="tanh", name=f"th_{gc}")
            nc.scalar.activation(out=th, in_=pa, func=AF.Tanh)
            if cc % 2 == 0:
                pv_cur = ps_att.tile([128, 512], F32, tag="psA",
                                     name=f"pv_{gc}")
            # previous chunk's vdot (tanh had a full chunk of slack)
            if pend_vdot is not None:
                emit_vdot(*pend_vdot)
            pend_vdot = (th, pv_cur, cc, t)
            # bank matmul pair for (t, j=cc) as PE filler
            j = cc
            for wh in range(2):
                bc = bc_pend.pop(0)
                gidx = (t * 4 + j) * 2 + wh
                if gidx + 2 < 64:
                    bc_pend.append(load_bc(gidx + 2))
                nc.tensor.matmul(
                    st["fpA"][32 * j:32 * (j + 1), :],
                    lhsT=st["mb"][:, 32 * (2 * j + wh):32 * (2 * j + wh + 1)],
                    rhs=bc[:, :],
                    start=(wh == 0), stop=(wh == 1),
                    tile_position=(0, 32 * j))
            if cc == 3:
                # bank reduction done -> transpose to featT
                bkA = p_mblk.tile([128, 256], F32, tag="bkA", name=f"bkA_{t}")
                nc.vector.tensor_copy(out=bkA, in_=st["fpA"])
                for h in range(2):
                    pmb = ps_bank.tile([128, 256], F32, tag="psB",
                                       name=f"pmb_{t}_{h}")
                    nc.tensor.transpose(pmb[0:128, 0:128],
                                        bkA[:, h * 128:(h + 1) * 128], cb_id)
                    fsb = p_featT.tile([128, 128], F32, tag="featT",
                                       name=f"fT_{t}_{h}")
                    nc.vector.tensor_copy(out=fsb, in_=pmb[0:128, 0:128])
                    featT_sb[t][h] = fsb
        emit_vdot(*pend_vdot)

        # ---- stage 5: score + agg coefficients (batched) ----
        sc = wpool.tile([128, 256], F32, name="sc")
        nc.vector.tensor_tensor(out=sc, in0=att_a, in1=ts_t, op=OP.add)
        sc2 = wpool.tile([128, 256], F32, name="sc2")
        nc.vector.tensor_scalar(out=sc2, in0=sc, scalar1=0.01, scalar2=None,
                                op0=OP.mult)
        nc.vector.tensor_tensor(out=sc, in0=sc, in1=sc2, op=OP.max)
        wgt = wpool.tile([128, 256], F32, name="wgt")
        nc.vector.tensor_tensor(out=wgt, in0=sc, in1=mrec, op=OP.mult)
        prod = wpool.tile([128, 256], F32, name="prod")
        for c, src in enumerate((alpha, beta, invn)):
            nc.vector.tensor_tensor(out=prod, in0=wgt, in1=src[:, 0:256], op=OP.mult)
            nc.vector.tensor_reduce(out=ABC[:, c * 8:(c + 1) * 8],
                                    in_=prod.rearrange("p (t k) -> p t k", k=K),
                                    axis=mybir.AxisListType.X, op=OP.add)

        # pack final rank-6 coefs: col = c*8 + t, rows: (as,bs,gs,A,B,C)
        packF = wpool.tile([128, 48], F32, name="packF")
        for c, src in ((0, alpha[:, 256:264]), (1, beta[:, 256:264]),
                       (2, invn[:, 256:264]), (3, ABC[:, 0:8]),
                       (4, ABC[:, 8:16]), (5, ABC[:, 16:24])):
            nc.vector.tensor_copy(out=packF[:, c * 8:(c + 1) * 8], in_=src)
        pmf = ps_att.tile([128, 512], F32, tag="psA", name="pm_packF")
        nc.tensor.transpose(pmf[0:48, 0:128], packF, cb_id)
        pFT = wpool.tile([48, 128], F32, name="pFT")
        nc.vector.tensor_copy(out=pFT, in_=pmf[0:48, 0:128])
        for c in range(6):
            nc.sync.dma_start(out=coefF6[c:c + 1, :],
                              in_=pFT[c * 8:(c + 1) * 8, :])

        # ---- stage 6: final matmuls + relu + store ----
        for t in range(NT):
            po = ps_att.tile([128, 512], F32, tag="psA", name=f"po_{t}")
            nc.tensor.matmul(po[:, 0:256], lhsT=featT_sb[t][0], rhs=cb_wT0,
                             start=True, stop=False)
            nc.tensor.matmul(po[:, 0:256], lhsT=featT_sb[t][1], rhs=cb_wT1,
                             start=False, stop=False)
            nc.tensor.matmul(po[:, 0:256], lhsT=coefF6[:, t * 128:(t + 1) * 128],
                             rhs=cb_b6h, start=False, stop=True)
            ot = p_out.tile([128, 256], F32, tag="outp", name=f"ot_{t}")
            nc.vector.tensor_scalar(out=ot, in0=po[:, 0:256], scalar1=0.0,
                                    scalar2=None, op0=OP.max)
            nc.sync.dma_start(out=d_out[t * 128:(t + 1) * 128, :], in_=ot)

    nc.compile()
    return nc


def _host_params(w_time, b_time, w_node, b_node, Wq, Wk, v_att, weight):
    f32 = np.float32
    w_time = np.asarray(w_time, f32)
    w_node = np.asarray(w_node, f32)
    bsum = np.asarray(b_time, f32) + np.asarray(b_node, f32)
    Wq = np.asarray(Wq, f32)
    Wk = np.asarray(Wk, f32)
    v = np.asarray(v_att, f32)
    weight = np.asarray(weight, f32)

    basis3 = np.stack([w_time, w_node, bsum])                  # [3, D]
    gram = basis3 @ basis3.T                                   # [3, 3]
    basis6att = np.zeros((6, D), f32)
    basis6att[0:3] = basis3 @ Wq
    basis6att[3:6] = basis3 @ Wk
    basis6H = np.zeros((6, H), f32)
    basis6H[0:3] = basis3 @ weight[:, :D].T
    basis6H[3:6] = basis3 @ weight[:, D:].T
    dmask = np.zeros((128, 32), f32)
    dmask[np.arange(128), np.arange(128) // 4] = 1.0
    return {
        "basis6att": basis6att.astype(ml_dtypes.bfloat16),
        "basis6H": basis6H,
        "v32": np.ascontiguousarray(np.tile(v.reshape(D, 1), (1, 32))),
        "weightT": np.ascontiguousarray(weight.T),
        "dmask": dmask,
        "ident": np.eye(128, dtype=f32),
        "gram": gram.astype(np.float64),
    }


def _perm_tk(x):
    # [EC,2,K] -> [128 p, (t k)]
    return np.ascontiguousarray(
        x.reshape(NT, 128, K).transpose(1, 0, 2).reshape(128, NT * K))


def _perm_t(x):
    # [EC,2] -> [128 p, t]
    return np.ascontiguousarray(x.reshape(NT, 128).T)


def _perm_bft(x):
    # [EC,2,W,D2] -> rows ((t j wh),(po wl)) x D2
    x = x.reshape(NT, 4, 32, 2, 4, D2)       # t j po wh wl d
    x = x.transpose(0, 1, 3, 2, 4, 5)        # t j wh po wl d
    return np.ascontiguousarray(x.reshape(64 * 128, D2))


def _expand_bank(x):
    # [EC,2,W] -> [128 (po,wl), 64 (t,j,wh)]: x[t*128+j*32+po, wh*4+wl]
    x = x.reshape(NT, 4, 32, 2, 4)          # t j po wh wl
    x = x.transpose(2, 4, 0, 1, 3)          # po wl t j wh
    return np.ascontiguousarray(x.reshape(128, 64))


def _shard_inputs(inputs):
    f32 = np.float32
    ins = []
    for c in range(NCORES):
        sl = slice(c * EC, (c + 1) * EC)
        ins.append({
            "dtn_p": _perm_tk(np.asarray(inputs["dt_neigh"][sl], f32)),
            "gcn_p": _perm_tk(np.asarray(inputs["gc_neigh"][sl], f32)),
            "mskn_p": _perm_tk(
                np.asarray(inputs["neigh_mask"][sl]).astype(f32)),
            "dts_p": _perm_t(np.asarray(inputs["dt_self"][sl], f32)),
            "gcs_p": _perm_t(np.asarray(inputs["gc_self"][sl], f32)),
            "bdt_e": _expand_bank(np.asarray(inputs["bank_dt"][sl], f32)),
            "bmsk_e": _expand_bank(
                np.asarray(inputs["bank_mask"][sl]).astype(f32)),
            "bft_p": _perm_bft(np.asarray(inputs["bank_feat"][sl], f32)),
        })
    return ins


_LAST_RESULT = {}


def kernel(**inputs):
    pp = _host_params(inputs["w_time"], inputs["b_time"], inputs["w_node"],
                      inputs["b_node"], inputs["Wq"], inputs["Wk"],
                      inputs["v_att"], inputs["weight"])
    nc = _build_program(pp)
    in_maps = _shard_inputs(inputs)
    import os
    trace = bool(int(os.environ.get("KBENCH_TRACE", "0")))
    res = run_bass_kernel_spmd(nc, in_maps, core_ids=list(range(NCORES)),
                               trace=trace)
    _LAST_RESULT["res"] = res
    outs = [res.results[c]["out"].reshape(EC, 2, H) for c in range(NCORES)]
    return np.ascontiguousarray(np.concatenate(outs, axis=0))


# revision 30
# speedup vs baseline: 1.6922x; 1.3220x over previous
"""Trainium2 Bass kernel for the gnn_message_passing problem.

Math (per edge e, side i):
  node_feat = l2norm(|dt|*w_time + b_time + gc*w_node + b_node)
  neigh_feat likewise per neighbor k
  att = tanh(node_feat@Wq + neigh_feat@Wk) . v_att
  score = leaky_relu(att + 2/(2+dt_neigh), 0.01)
  agg = sum_k (score*mask/n_neigh) * neigh_feat
  combined = [node_feat, agg]
  feat = sum_w exp(-0.5*bank_dt)*bank_mask * bank_feat + combined
  out = relu(feat @ weight.T)

Key structure exploited: every featurized vector lies in span{w_time, w_node,
b_time+b_node}, so node/neigh features are 3 scalars each. q+kk collapses to a
rank-6 combination of 6 fixed D-vectors; the "combined @ W.T" part of the
output collapses to a rank-6 combination of 6 fixed H-vectors. Only the
tanh( . ) . v contraction (E*2*K*D tanh evals) and the bank-feature reduction
touch O(E*K*D)-sized data on-device.

Sharding: pure data-parallel over E across 8 cores (one SPMD program).
"""

import numpy as np
import ml_dtypes

import concourse.bass as bass
import concourse.bacc as bacc
import concourse.mybir as mybir
import concourse.tile as tile
from concourse.bass_utils import run_bass_kernel_spmd

F32 = mybir.dt.float32
BF16 = mybir.dt.bfloat16
AF = mybir.ActivationFunctionType
OP = mybir.AluOpType

E, K, W, D, H = 4096, 32, 8, 128, 256
NCORES = 8
EC = E // NCORES          # 512 edges per core
POS = EC * 2              # 1024 (edge, side) positions per core
NT = POS // 128           # 8 position tiles of 128
D2 = 2 * D                # 256
CHUNKS = 4                # tanh chunks of 1024 cols per tile
VLAG = 2                  # vdot trails arg/tanh by 2 chunks


def _build_program(pp):
    """Build the SPMD single-core program. pp: dict of host-precomputed params."""
    nc = bacc.Bacc("TRN2", target_bir_lowering=False, debug=False)

    # ---- DRAM I/O (per core shard), host-prepermuted layouts ----
    d_dtn = nc.dram_tensor("dtn_p", [128, 256], F32, kind="ExternalInput")
    d_gcn = nc.dram_tensor("gcn_p", [128, 256], F32, kind="ExternalInput")
    d_msk = nc.dram_tensor("mskn_p", [128, 256], F32, kind="ExternalInput")
    d_dts = nc.dram_tensor("dts_p", [128, 8], F32, kind="ExternalInput")
    d_gcs = nc.dram_tensor("gcs_p", [128, 8], F32, kind="ExternalInput")
    d_bdt = nc.dram_tensor("bdt_e", [128, 64], F32, kind="ExternalInput")
    d_bmsk = nc.dram_tensor("bmsk_e", [128, 64], F32, kind="ExternalInput")
    # chunk-contiguous bf16: chunk c=(t*4+j)*2+wh -> rows c*128..(c+1)*128
    d_bft = nc.dram_tensor("bft_p", [64 * 128, D2], F32, kind="ExternalInput")
    d_out = nc.dram_tensor("out", [POS, H], F32, kind="ExternalOutput")

    # ---- inline constants ----
    c_basis = nc.inline_tensor(pp["basis6att"], name="c_basis")    # [6,128] bf16
    c_b6h = nc.inline_tensor(pp["basis6H"], name="c_b6h")          # [6,256] f32
    c_v = nc.inline_tensor(pp["v32"], name="c_v")                  # [128,32] f32
    c_wT = nc.inline_tensor(pp["weightT"], name="c_wT")            # [256,256] f32
    c_dmask = nc.inline_tensor(pp["dmask"], name="c_dmask")        # [128,32] f32
    c_ident = nc.inline_tensor(pp["ident"], name="c_ident")        # [128,128] f32
    G = pp["gram"]  # 3x3 float

    from contextlib import ExitStack
    with tile.TileContext(nc) as tc, ExitStack() as ctx:
        cpool = ctx.enter_context(tc.tile_pool(name="consts", bufs=1))
        wpool = ctx.enter_context(tc.tile_pool(name="work", bufs=1))
        p_coef6 = ctx.enter_context(tc.tile_pool(name="coef6", bufs=2))
        p_tanh = ctx.enter_context(tc.tile_pool(name="tanh", bufs=4))
        p_attT = ctx.enter_context(tc.tile_pool(name="attT", bufs=3))
        p_featT = ctx.enter_context(tc.tile_pool(name="featT", bufs=4))
        p_bch = ctx.enter_context(tc.tile_pool(name="bch", bufs=6))
        p_mblk = ctx.enter_context(tc.tile_pool(name="mblk", bufs=2))
        p_bankC = ctx.enter_context(tc.tile_pool(name="bankC", bufs=8))
        p_out = ctx.enter_context(tc.tile_pool(name="outp", bufs=2))
        ps_arg = ctx.enter_context(tc.tile_pool(name="ps_arg", bufs=2, space="PSUM"))
        ps_mix = ctx.enter_context(tc.tile_pool(name="ps_mix", bufs=4, space="PSUM"))

        # ---- constants to SBUF ----
        cb_basis = cpool.tile([6, 128], BF16, name="cb_basis")
        nc.sync.dma_start(out=cb_basis, in_=c_basis[:, :])
        cb_b6h = cpool.tile([6, 256], F32, name="cb_b6h")
        nc.sync.dma_start(out=cb_b6h, in_=c_b6h[:, :])
        cb_v = cpool.tile([128, 32], F32, name="cb_v")
        nc.sync.dma_start(out=cb_v, in_=c_v[:, :])
        cb_wT0 = cpool.tile([128, 256], F32, name="cb_wT0")
        nc.sync.dma_start(out=cb_wT0, in_=c_wT[0:128, :])
        cb_wT1 = cpool.tile([128, 256], F32, name="cb_wT1")
        nc.sync.dma_start(out=cb_wT1, in_=c_wT[128:256, :])
        cb_dmask = cpool.tile([128, 32], F32, name="cb_dmask")
        nc.sync.dma_start(out=cb_dmask, in_=c_dmask[:, :])
        cb_id = cpool.tile([128, 128], F32, name="cb_id")
        nc.sync.dma_start(out=cb_id, in_=c_ident[:, :])

        # ---- loads ----
        bdt_e = wpool.tile([128, 64], F32, name="bdt_e")
        bmsk_e = wpool.tile([128, 64], F32, name="bmsk_e")
        nc.sync.dma_start(out=bdt_e, in_=d_bdt[:, :])
        nc.sync.dma_start(out=bmsk_e, in_=d_bmsk[:, :])
        t_dtn = wpool.tile([128, 256], F32, name="t_dtn")
        nc.sync.dma_start(out=t_dtn[:, :], in_=d_dtn[:, :])
        t_m = wpool.tile([128, 256], F32, name="t_m")
        nc.sync.dma_start(out=t_m[:, :], in_=d_msk[:, :])
        a_all = wpool.tile([128, 264], F32, name="a_all")
        b_all = wpool.tile([128, 264], F32, name="b_all")
        nc.sync.dma_start(out=a_all[:, 0:256], in_=d_dtn[:, :])
        nc.sync.dma_start(out=a_all[:, 256:264], in_=d_dts[:, :])
        nc.sync.dma_start(out=b_all[:, 0:256], in_=d_gcn[:, :])
        nc.sync.dma_start(out=b_all[:, 256:264], in_=d_gcs[:, :])

        # ---- bank decay weights first (ACT exp before sqrt: unblocks bank
        # pipeline; costs one extra table load, hidden early) ----
        bwe = wpool.tile([128, 64], F32, name="bwe")
        nc.scalar.activation(out=bwe, in_=bdt_e, func=AF.Exp, scale=-0.5)
        nc.vector.tensor_tensor(out=bwe, in0=bwe, in1=bmsk_e, op=OP.mult)

        # ---- featurize scalars ----
        nega = wpool.tile([128, 264], F32, name="nega")
        nc.vector.tensor_scalar(out=nega, in0=a_all, scalar1=-1.0, scalar2=None,
                                op0=OP.mult)
        nc.vector.tensor_tensor(out=a_all, in0=a_all, in1=nega, op=OP.max)
        aa = wpool.tile([128, 264], F32, name="aa")
        ab = wpool.tile([128, 264], F32, name="ab")
        bb = wpool.tile([128, 264], F32, name="bb")
        nc.vector.tensor_tensor(out=aa, in0=a_all, in1=a_all, op=OP.mult)
        nc.vector.tensor_tensor(out=ab, in0=a_all, in1=b_all, op=OP.mult)
        nc.vector.tensor_tensor(out=bb, in0=b_all, in1=b_all, op=OP.mult)
        n2 = wpool.tile([128, 264], F32, name="n2")
        nc.vector.tensor_scalar(out=n2, in0=aa, scalar1=float(G[0, 0]),
                                scalar2=float(G[2, 2]), op0=OP.mult, op1=OP.add)
        nc.vector.scalar_tensor_tensor(out=n2, in0=bb, scalar=float(G[1, 1]),
                                       in1=n2, op0=OP.mult, op1=OP.add)
        nc.vector.scalar_tensor_tensor(out=n2, in0=a_all, scalar=float(2 * G[0, 2]),
                                       in1=n2, op0=OP.mult, op1=OP.add)
        nc.vector.scalar_tensor_tensor(out=n2, in0=b_all, scalar=float(2 * G[1, 2]),
                                       in1=n2, op0=OP.mult, op1=OP.add)
        nc.vector.scalar_tensor_tensor(out=n2, in0=ab, scalar=float(2 * G[0, 1]),
                                       in1=n2, op0=OP.mult, op1=OP.add)
        nrm = wpool.tile([128, 264], F32, name="nrm")
        nc.scalar.activation(out=nrm, in_=n2, func=AF.Sqrt)
        nc.vector.tensor_scalar(out=nrm, in0=nrm, scalar1=1e-12, scalar2=None,
                                op0=OP.max)
        scr = wpool.tile([128, 264], F32, name="scr")
        invn = wpool.tile([128, 264], F32, name="invn")
        nc.vector.reciprocal_approx_accurate(out=invn, in_=nrm, scratch=scr)
        alpha = wpool.tile([128, 264], F32, name="alpha")
        beta = wpool.tile([128, 264], F32, name="beta")
        nc.vector.tensor_tensor(out=alpha, in0=a_all, in1=invn, op=OP.mult)
        nc.vector.tensor_tensor(out=beta, in0=b_all, in1=invn, op=OP.mult)

        # time decay 2/(2+dt) on raw dt
        ts_t = wpool.tile([128, 256], F32, name="ts_t")
        scr2 = wpool.tile([128, 256], F32, name="scr2")
        nc.vector.tensor_scalar(out=ts_t, in0=t_dtn, scalar1=2.0, scalar2=None,
                                op0=OP.add)
        nc.vector.reciprocal_approx_accurate(out=ts_t, in_=ts_t, scratch=scr2)
        nc.vector.tensor_scalar(out=ts_t, in0=ts_t, scalar1=2.0, scalar2=None,
                                op0=OP.mult)

        # n_neigh and mask/n_neigh
        nn = wpool.tile([128, 8], F32, name="nn")
        nc.vector.tensor_reduce(out=nn, in_=t_m.rearrange("p (t k) -> p t k", k=K),
                                axis=mybir.AxisListType.X, op=OP.add)
        nc.vector.tensor_scalar(out=nn, in0=nn, scalar1=1.0, scalar2=None,
                                op0=OP.max)
        innn = wpool.tile([128, 8], F32, name="innn")
        scr3 = wpool.tile([128, 8], F32, name="scr3")
        nc.vector.reciprocal_approx_accurate(out=innn, in_=nn, scratch=scr3)
        mrec = wpool.tile([128, 256], F32, name="mrec")
        nc.vector.tensor_tensor(
            out=mrec.rearrange("p (t k) -> p t k", k=K),
            in0=t_m.rearrange("p (t k) -> p t k", k=K),
            in1=innn.unsqueeze(2).broadcast_to([128, 8, K]), op=OP.mult)

        # ---- transposes for coef rows ----
        packS = wpool.tile([128, 24], F32, name="packS")
        nc.vector.tensor_copy(out=packS[:, 0:8], in_=alpha[:, 256:264])
        nc.vector.tensor_copy(out=packS[:, 8:16], in_=beta[:, 256:264])
        nc.vector.tensor_copy(out=packS[:, 16:24], in_=invn[:, 256:264])
        pm = ps_mix.tile([128, 512], F32, tag="mix", name="pm_selfT")
        nc.tensor.transpose(pm[0:24, 0:128], packS, cb_id)
        selfT = wpool.tile([32, 128], BF16, name="selfT")
        nc.vector.tensor_copy(out=selfT[0:24, :], in_=pm[0:24, 0:128])

        coefT = []
        for (nm, srcT) in (("aT", alpha), ("bT", beta), ("gT", invn)):
            halves = []
            for h in range(2):
                pmx = ps_mix.tile([128, 512], F32, tag="mix", name=f"pm_{nm}{h}")
                nc.tensor.transpose(pmx[0:128, 0:128],
                                    srcT[:, h * 128:(h + 1) * 128], cb_id)
                sb = wpool.tile([128, 128], BF16, name=f"{nm}{h}")
                nc.vector.tensor_copy(out=sb, in_=pmx[0:128, 0:128])
                halves.append(sb)
            coefT.append(halves)

        att_a = wpool.tile([128, 256], F32, name="att_a")
        coefF6 = wpool.tile([6, 8 * 128], F32, name="coefF6")
        ABC = wpool.tile([128, 24], F32, name="ABC")  # cols c*8+t
        bankC_sb = [None] * NT

        # ---- helpers ----
        def build_coef6(t):
            c6 = p_coef6.tile([6, 4096], BF16, tag="coef6", name=f"coef6_{t}")
            for c in range(3):
                r = c * 8 + t
                nc.sync.dma_start(
                    out=c6[c:c + 1, :],
                    in_=selfT[r:r + 1, :].unsqueeze(1).broadcast_to([1, K, 128]))
            for c in range(3):
                src = coefT[c][t // 4]
                nc.sync.dma_start(
                    out=c6[3 + c:4 + c, :],
                    in_=src[(t % 4) * 32:(t % 4) * 32 + 32, :])
            return c6

        def build_mb(t):
            mb = p_mblk.tile([128, 256], F32, tag="mblk", name=f"mb_{t}")
            nc.vector.tensor_tensor(
                out=mb.rearrange("r (b c) -> r b c", c=32),
                in0=cb_dmask.unsqueeze(1).broadcast_to([128, 8, 32]),
                in1=bwe[:, t * 8:(t + 1) * 8].unsqueeze(2).broadcast_to(
                    [128, 8, 32]),
                op=OP.mult)
            return mb

        def load_bc(gidx):
            bc = p_bch.tile([128, 256], F32, tag="bch", name=f"bc_{gidx}")
            nc.gpsimd.dma_start(out=bc[:, :],
                                in_=d_bft[gidx * 128:(gidx + 1) * 128, :])
            return bc

        coef6_t = build_coef6(0)
        mb_t = build_mb(0)
        bc_pend = [load_bc(0), load_bc(1), load_bc(2), load_bc(3)]
        state = {}
        pend = []               # [(th, cc, t)] vdots not yet emitted
        pv_by_group = {}

        def emit_vdot(th, cc, t):
            g = (t * CHUNKS + cc) // 2
            if cc % 2 == 0:
                pv_by_group[g] = ps_mix.tile([128, 512], F32, tag="mix",
                                             name=f"pv_{g}")
            pv = pv_by_group[g]
            for mm in range(2):
                q = (cc % 2) * 2 + mm
                nc.tensor.matmul(pv[32 * q:32 * (q + 1), :], lhsT=cb_v,
                                 rhs=th[:, mm * 512:(mm + 1) * 512],
                                 start=True, stop=True,
                                 tile_position=(0, 32 * q))
            if cc % 2 == 1:
                b = cc // 2
                ast = p_mblk.tile([128, 512], F32, tag="astage",
                                  name=f"ast_{t}_{cc}")
                nc.vector.tensor_copy(out=ast[:, :], in_=pv[:, :])
                attT = state[t]["attT"]
                nc.sync.dma_start(
                    out=attT[16 * b:16 * (b + 1), :],
                    in_=ast.rearrange("(q r) (kl p) -> q r kl p",
                                      r=32, p=128)[:, 0])
            if cc == 3:
                attT = state[t]["attT"]
                pmx = ps_mix.tile([128, 512], F32, tag="mix", name=f"pmxa_{t}")
                nc.tensor.transpose(pmx[0:128, 0:32], attT, cb_id[0:32, 0:32])
                nc.vector.tensor_copy(out=att_a[:, 32 * t:32 * (t + 1)],
                                      in_=pmx[0:128, 0:32])

        # ---- software-pipelined global chunk loop ----
        for gc in range(NT * CHUNKS):
            t, cc = divmod(gc, CHUNKS)
            if cc == 0:
                state[t] = {
                    "attT": p_attT.tile([32, 128], F32, tag="attT",
                                        name=f"attT_{t}"),
                    "fpA": ps_mix.tile([128, 512], F32, tag="mix",
                                       name=f"fpA_{t}"),
                    "coef6": coef6_t, "mb": mb_t,
                }
                if t + 1 < NT:
                    coef6_t = build_coef6(t + 1)
                    mb_t = build_mb(t + 1)
            st = state[t]
            pa = ps_arg.tile([128, 1024], F32, tag="psarg", name=f"pa_{gc}")
            for mm in range(2):
                nc.tensor.matmul(
                    pa[:, mm * 512:(mm + 1) * 512], lhsT=cb_basis,
                    rhs=st["coef6"][:, cc * 1024 + mm * 512:
                                    cc * 1024 + (mm + 1) * 512],
                    start=True, stop=True)
            th = p_tanh.tile([128, 1024], F32, tag="tanh", name=f"th_{gc}")
            nc.scalar.activation(out=th, in_=pa, func=AF.Tanh)
            if len(pend) >= VLAG:
                emit_vdot(*pend.pop(0))
            pend.append((th, cc, t))
            # bank matmul pair for (t, j=cc) as PE filler
            j = cc
            for wh in range(2):
                bc = bc_pend.pop(0)
                gidx = (t * 4 + j) * 2 + wh
                if gidx + 4 < 64:
                    bc_pend.append(load_bc(gidx + 4))
                nc.tensor.matmul(
                    st["fpA"][32 * j:32 * (j + 1), 0:256],
                    lhsT=st["mb"][:, 32 * (2 * j + wh):32 * (2 * j + wh + 1)],
                    rhs=bc[:, :],
                    start=(wh == 0), stop=(wh == 1),
                    tile_position=(0, 32 * j))
            if cc == 3:
                # bank reduction -> featT -> inline bank-part of output matmul
                bkA = p_mblk.tile([128, 256], F32, tag="bkA", name=f"bkA_{t}")
                nc.vector.tensor_copy(out=bkA, in_=st["fpA"][:, 0:256])
                fsb = [None, None]
                for h in range(2):
                    pmb = ps_mix.tile([128, 512], F32, tag="mix",
                                      name=f"pmb_{t}_{h}")
                    nc.tensor.transpose(pmb[0:128, 0:128],
                                        bkA[:, h * 128:(h + 1) * 128], cb_id)
                    fsb[h] = p_featT.tile([128, 128], F32, tag="featT",
                                          name=f"fT_{t}_{h}")
                    nc.vector.tensor_copy(out=fsb[h], in_=pmb[0:128, 0:128])
                poB = ps_mix.tile([128, 512], F32, tag="mix", name=f"poB_{t}")
                nc.tensor.matmul(poB[:, 0:256], lhsT=fsb[0], rhs=cb_wT0,
                                 start=True, stop=False)
                nc.tensor.matmul(poB[:, 0:256], lhsT=fsb[1], rhs=cb_wT1,
                                 start=False, stop=True)
                bankC_sb[t] = p_bankC.tile([128, 256], F32, tag="bankC",
                                           name=f"bankC_{t}")
                nc.vector.tensor_copy(out=bankC_sb[t], in_=poB[:, 0:256])
        while pend:
            emit_vdot(*pend.pop(0))

        # ---- score + agg coefficients (batched) ----
        sc = wpool.tile([128, 256], F32, name="sc")
        nc.vector.tensor_tensor(out=sc, in0=att_a, in1=ts_t, op=OP.add)
        sc2 = wpool.tile([128, 256], F32, name="sc2")
        nc.vector.tensor_scalar(out=sc2, in0=sc, scalar1=0.01, scalar2=None,
                                op0=OP.mult)
        nc.vector.tensor_tensor(out=sc, in0=sc, in1=sc2, op=OP.max)
        wgt = wpool.tile([128, 256], F32, name="wgt")
        nc.vector.tensor_tensor(out=wgt, in0=sc, in1=mrec, op=OP.mult)
        prod = wpool.tile([128, 256], F32, name="prod")
        for c, src in enumerate((alpha, beta, invn)):
            nc.vector.tensor_tensor(out=prod, in0=wgt, in1=src[:, 0:256],
                                    op=OP.mult)
            nc.vector.tensor_reduce(out=ABC[:, c * 8:(c + 1) * 8],
                                    in_=prod.rearrange("p (t k) -> p t k", k=K),
                                    axis=mybir.AxisListType.X, op=OP.add)

        # pack final rank-6 coefs: col = c*8 + t, rows: (as,bs,gs,A,B,C)
        packF = wpool.tile([128, 48], F32, name="packF")
        for c, src in ((0, alpha[:, 256:264]), (1, beta[:, 256:264]),
                       (2, invn[:, 256:264]), (3, ABC[:, 0:8]),
                       (4, ABC[:, 8:16]), (5, ABC[:, 16:24])):
            nc.vector.tensor_copy(out=packF[:, c * 8:(c + 1) * 8], in_=src)
        pmf = ps_mix.tile([128, 512], F32, tag="mix", name="pm_packF")
        nc.tensor.transpose(pmf[0:48, 0:128], packF, cb_id)
        pFT = wpool.tile([48, 128], F32, name="pFT")
        nc.vector.tensor_copy(out=pFT, in_=pmf[0:48, 0:128])
        for c in range(6):
            nc.sync.dma_start(out=coefF6[c:c + 1, :],
                              in_=pFT[c * 8:(c + 1) * 8, :])

        # ---- tail: rank-6 combined part + add + relu + store ----
        for t in range(NT):
            pc = ps_mix.tile([128, 512], F32, tag="mix", name=f"pc_{t}")
            nc.tensor.matmul(pc[:, 0:256], lhsT=coefF6[:, t * 128:(t + 1) * 128],
                             rhs=cb_b6h, start=True, stop=True)
            ot = p_out.tile([128, 256], F32, tag="outp", name=f"ot_{t}")
            nc.vector.tensor_tensor(out=ot, in0=pc[:, 0:256], in1=bankC_sb[t],
                                    op=OP.add)
            nc.vector.tensor_scalar(out=ot, in0=ot, scalar1=0.0, scalar2=None,
                                    op0=OP.max)
            nc.sync.dma_start(out=d_out[t * 128:(t + 1) * 128, :], in_=ot)

    nc.compile()
    return nc


def _host_params(w_time, b_time, w_node, b_node, Wq, Wk, v_att, weight):
    f32 = np.float32
    w_time = np.asarray(w_time, f32)
    w_node = np.asarray(w_node, f32)
    bsum = np.asarray(b_time, f32) + np.asarray(b_node, f32)
    Wq = np.asarray(Wq, f32)
    Wk = np.asarray(Wk, f32)
    v = np.asarray(v_att, f32)
    weight = np.asarray(weight, f32)

    basis3 = np.stack([w_time, w_node, bsum])                  # [3, D]
    gram = basis3 @ basis3.T
    basis6att = np.zeros((6, D), f32)
    basis6att[0:3] = basis3 @ Wq
    basis6att[3:6] = basis3 @ Wk
    basis6H = np.zeros((6, H), f32)
    basis6H[0:3] = basis3 @ weight[:, :D].T
    basis6H[3:6] = basis3 @ weight[:, D:].T
    dmask = np.zeros((128, 32), f32)
    dmask[np.arange(128), np.arange(128) // 4] = 1.0
    return {
        "basis6att": basis6att.astype(ml_dtypes.bfloat16),
        "basis6H": basis6H,
        "v32": np.ascontiguousarray(np.tile(v.reshape(D, 1), (1, 32))),
        "weightT": np.ascontiguousarray(weight.T),
        "dmask": dmask,
        "ident": np.eye(128, dtype=f32),
        "gram": gram.astype(np.float64),
    }


def _perm_tk(x):
    # [EC,2,K] -> [128 p, (t k)]
    return np.ascontiguousarray(
        x.reshape(NT, 128, K).transpose(1, 0, 2).reshape(128, NT * K))


def _perm_t(x):
    # [EC,2] -> [128 p, t]
    return np.ascontiguousarray(x.reshape(NT, 128).T)


def _perm_bft(x):
    # [EC,2,W,D2] -> rows ((t j wh),(po wl)) x D2, bf16
    x = x.reshape(NT, 4, 32, 2, 4, D2)       # t j po wh wl d
    x = x.transpose(0, 1, 3, 2, 4, 5)        # t j wh po wl d
    return np.ascontiguousarray(x.reshape(64 * 128, D2))


def _expand_bank(x):
    # [EC,2,W] -> [128 (po,wl), 64 (t,j,wh)]: x[t*128+j*32+po, wh*4+wl]
    x = x.reshape(NT, 4, 32, 2, 4)          # t j po wh wl
    x = x.transpose(2, 4, 0, 1, 3)          # po wl t j wh
    return np.ascontiguousarray(x.reshape(128, 64))


def _shard_inputs(inputs):
    f32 = np.float32
    ins = []
    for c in range(NCORES):
        sl = slice(c * EC, (c + 1) * EC)
        ins.append({
            "dtn_p": _perm_tk(np.asarray(inputs["dt_neigh"][sl], f32)),
            "gcn_p": _perm_tk(np.asarray(inputs["gc_neigh"][sl], f32)),
            "mskn_p": _perm_tk(
                np.asarray(inputs["neigh_mask"][sl]).astype(f32)),
            "dts_p": _perm_t(np.asarray(inputs["dt_self"][sl], f32)),
            "gcs_p": _perm_t(np.asarray(inputs["gc_self"][sl], f32)),
            "bdt_e": _expand_bank(np.asarray(inputs["bank_dt"][sl], f32)),
            "bmsk_e": _expand_bank(
                np.asarray(inputs["bank_mask"][sl]).astype(f32)),
            "bft_p": _perm_bft(np.asarray(inputs["bank_feat"][sl], f32)),
        })
    return ins


_LAST_RESULT = {}


def kernel(**inputs):
    pp = _host_params(inputs["w_time"], inputs["b_time"], inputs["w_node"],
                      inputs["b_node"], inputs["Wq"], inputs["Wk"],
                      inputs["v_att"], inputs["weight"])
    nc = _build_program(pp)
    in_maps = _shard_inputs(inputs)
    import os
    trace = bool(int(os.environ.get("KBENCH_TRACE", "0")))
    res = run_bass_kernel_spmd(nc, in_maps, core_ids=list(range(NCORES)),
                               trace=trace)
    _LAST_RESULT["res"] = res
    outs = [res.results[c]["out"].reshape(EC, 2, H) for c in range(NCORES)]
    return np.ascontiguousarray(np.concatenate(outs, axis=0))
